# revision 37
# baseline (speedup 1.0000x reference)
"""DensityLoss kernel for 8x Trainium2 NeuronCores (raw Bass), bf16 I/O.

out[b,y,x] = loss[b,y,x] * (10 if covered by any bbox else 1) / (H*W*B)

Tolerance is rel 2e-2, so the device streams bf16: the host pre-scales
loss by 10*s (s = 1/(H*W*B) = 2**-23; 10*s = 1.25*2^-20, exact) and
casts to bf16; the device computes ot = fw * lt3 with the weight
encoded as fw in {0.1, 1.0} (bg, fg), and the host upcasts the bf16
result. Total rel err ~0.5% (two bf16 roundings + the 0.1 encoding).

Coverage count is separable:
    count[y,x] = sum_n rowmask_n[y] * colmask_n[x]
i.e. a [128,64]x[64,512] matmul per tile on the PE (bf16 0/1
indicators, exact integer counts in PSUM f32). Masks are built
128-partition-packed: partition 64h+n holds box n's indicator for
coordinate half h (thresholds shifted by -512 for h=1), so the six big
DVE compare/product ops run [128,512] at full lane count. Matmul tiles
for row-half h address base partition 64h for BOTH operands (PE
quadrant tile_position), with the column masks duplicated into both
partition halves: C0 = x in [0,512) twice, C1 = x in [512,1024) twice.

The count->weight conversion is the compute bottleneck (1M PSUM f32
values must cross to SBUF through DVE/ACT), so it is split per-tile:
  even tiles: ACT  fw = Sigmoid(10*cnt - ln 9)  -> {~0.1, 1.0} bf16
  odd tiles:  DVE  fw = max(min(cnt, 1), 0.1)   -> {0.1, 1.0} bf16
(sigmoid(-ln 9) = 1/10 exactly; sigmoid(>=7.8) rounds to 1.0 in bf16.)
The 1M bf16 multiplies split likewise: even-tile mults on GpSimd, odd
on DVE. No ACT Copy pass, no intermediate u buffer.

Both lt3 (loss) and ot are parity-double-buffered on the rep index:
rep r uses buffer r%2 and waits only on rep r-2's consumers, keeping
DMA-completion receipts (~1-2us) off the steady-state critical path.

Data-parallel over batch: one image per NeuronCore, no collectives.
pred_densities is unused by the reference math.

Raw Bass (not Tile): the walrus build in this container allows at most
one semaphore wait attached per instruction and rejects Tile's
kernel-tail drain, so synchronization is explicit standalone waits.

Per core, NT=16 tiles of [128 rows, 512 cols] (idx = 2*m + n2):
  sync:   bf16 loss chunk loads -> lt3[a][r%2]   (HWDGE ring 1)
  PE:     cnt[g%8] = Rp[..].T @ C{n2}[..]        (PSUM, 8 banks)
  ACT:    even-tile fw (Sigmoid); chunk stores   (HWDGE ring 2)
  DVE:    mask prep; odd-tile fw; odd-tile mults
  gpsimd: bbox DMAs, iota; even-tile mults
"""

from contextlib import ExitStack

import numpy as np

import concourse.bass as bass
import concourse.mybir as mybir
from concourse.bass_utils import run_bass_kernel_spmd

B, H, W, N = 8, 1024, 1024, 64
P = 128            # SBUF partitions
NF = 512           # matmul free-dim tile (one PSUM bank of f32)
TM = 4             # row-tiles per DMA chunk (chunk = [128, 4096] bf16 = 1MB)
NT = 16            # total [128,512] tiles per image
BALANCE = 10.0
SCALE = 1.0 / float(H * W * B)  # 2**-23, exact power of two
PRESCALE = BALANCE * SCALE      # 1.25 * 2**-20, exact
LN9 = 2.1972246    # sigmoid(-ln 9) = 1/10

F32 = mybir.dt.float32
BF16 = mybir.dt.bfloat16
I32 = mybir.dt.int32


def build_program(repeat=1, tm=TM, dma_only=False, compute_only=False,
                  pe_only=False, merge_mm=False):
    """repeat>1 re-runs the whole pipeline on the same data inside one
    NEFF (for wall-clock HW timing via differencing). Global tile index
    g = rep*NT + idx drives all modular slot reuse and sem counts.

    dma_only: loads + stores only (stores read lt3 directly), no compute
    — isolates the DMA stream rate. compute_only: loads/stores emitted
    for rep 0 only — isolates the PE/DVE/ACT/GpSimd pipeline rate. Both
    are timing probes; output is garbage for repeat>1.
    """
    R = repeat

    def _chunk(idx):      # which DMA chunk a tile belongs to
        return idx // (2 * tm)

    def _cols(idx):       # free-dim slice inside the chunk's [128, tm*W]
        m, n2 = idx // 2, idx % 2
        lo = (m % tm) * W + n2 * NF
        return slice(lo, lo + NF)

    nc = bass.Bass()
    loss = nc.dram_tensor("loss", [H, W], BF16, kind="ExternalInput")
    bboxes = nc.dram_tensor("bboxes", [N, 4], I32, kind="ExternalInput")
    out = nc.dram_tensor("out", [H, W], BF16, kind="ExternalOutput")

    loss_v = loss[:].rearrange("(a t p) w -> a p t w", t=tm, p=P)
    out_v = out[:].rearrange("(a t p) w -> a p t w", t=tm, p=P)
    nchunks = H // (tm * P)
    tpc = 2 * tm             # tiles per chunk

    ge = mybir.AluOpType.is_ge
    lt_op = mybir.AluOpType.is_lt
    add = mybir.AluOpType.add
    mult = mybir.AluOpType.mult
    min_op = mybir.AluOpType.min
    max_op = mybir.AluOpType.max

    # mult-done sem counts by route: even-tile mults inc s_ttg (GpSimd),
    # odd-tile mults inc s_tt (DVE). Both count in tile order per engine.
    def n_even(upto):     # number of even tiles with index < upto
        return (upto + 1) // 2

    def n_odd(upto):
        return upto // 2

    with ExitStack() as ctx:
        en = ctx.enter_context
        # SBUF
        bb = en(nc.sbuf_tensor("bb", [P, 4], I32))      # bboxes, duplicated
        bbfA = en(nc.sbuf_tensor("bbfA", [P, 4], F32))  # f32(bb)
        bbfS = en(nc.sbuf_tensor("bbfS", [P, 4], F32))  # f32(bb) - 512
        bbfR = en(nc.sbuf_tensor("bbfR", [P, 4], F32))  # lo: A, hi: S
        iof = en(nc.sbuf_tensor("iof", [P, NF], F32))   # 0..511 each part.
        rge = en(nc.sbuf_tensor("rge", [P, NF], BF16))
        rlt = en(nc.sbuf_tensor("rlt", [P, NF], BF16))
        Rp = en(nc.sbuf_tensor("Rp", [P, NF], BF16))    # row masks, packed
        cge = en(nc.sbuf_tensor("cge", [P, NF], BF16))
        clt = en(nc.sbuf_tensor("clt", [P, NF], BF16))
        cge1 = en(nc.sbuf_tensor("cge1", [P, NF], BF16))
        clt1 = en(nc.sbuf_tensor("clt1", [P, NF], BF16))
        # col masks, both x-halves side by side in the free dim (and each
        # duplicated into both partition halves): CC[:, 0:NF] = x<512,
        # CC[:, NF:] = x>=512. Lets one matmul take a [64, 2*NF] rhs.
        CC = en(nc.sbuf_tensor("CC", [P, 2 * NF], BF16))
        lt3 = [[en(nc.sbuf_tensor(f"lt{a}_{q}", [P, tm * W], BF16))
                for q in range(2)] for a in range(nchunks)]
        ot = [[en(nc.sbuf_tensor(f"ot{a}_{q}", [P, tm * W], BF16))
               for q in range(2)] for a in range(nchunks)]
        fw = en(nc.sbuf_tensor("fw", [P, 8 * NF], BF16))   # 8 slots
        warm = en(nc.sbuf_tensor("warm", [P, 1], F32))
        biasT = en(nc.sbuf_tensor("biasT", [P, 1], F32))   # -ln 9
        # PSUM as 4 bank-pairs: one merged matmul writes a [P, 2*NF] pair;
        # tile g's counts live at cnt_ap(g) (bank g%8 either way).
        cntP = [en(nc.psum_tensor(f"cnt{i}", [P, 2 * NF], F32))
                for i in range(4)]

        def cnt_ap(g):
            return cntP[(g % 8) // 2][:, (g % 2) * NF:(g % 2 + 1) * NF]
        # semaphores
        s_bb = en(nc.semaphore("s_bb"))
        s_io = en(nc.semaphore("s_io"))
        s_prep = en(nc.semaphore("s_prep"))
        s_ld = [[en(nc.semaphore(f"s_ld{a}_{q}")) for q in range(2)]
                for a in range(nchunks)]
        s_mm = en(nc.semaphore("s_mm"))
        s_f = en(nc.semaphore("s_f"))    # even-tile fw (ACT)
        s_fd = en(nc.semaphore("s_fd"))  # odd-tile fw (DVE)
        s_tt = en(nc.semaphore("s_tt"))    # odd-tile mults (DVE)
        s_ttg = en(nc.semaphore("s_ttg"))  # even-tile mults (GpSimd)
        s_st = [[en(nc.semaphore(f"s_st{a}_{q}")) for q in range(2)]
                for a in range(nchunks)]

        block = en(nc.Block())

        def make_waiter(eng):
            """wait_ge with dominated-wait elision: once this engine has
            waited sem >= v, any later wait sem >= v' <= v is a no-op
            (sem values are monotone), so skip emitting it."""
            seen = {}
            def w(sem, val):
                k = id(sem)
                if seen.get(k, -1) < val:
                    seen[k] = val
                    eng.wait_ge(sem, val)
            return w

        def mult_done_waits(w, upto):
            """Wait until all multiplies for tiles < upto completed."""
            if n_even(upto):
                w(s_ttg, n_even(upto))
            if n_odd(upto):
                w(s_tt, n_odd(upto))

        def fw_done_wait(w, g):
            """Wait until fw for global tile g was produced."""
            if g % 2 == 0:
                w(s_f, g // 2 + 1)
            else:
                w(s_fd, g // 2 + 1)

        @block.sync
        def _(sync):
            w = make_waiter(sync)
            if pe_only:
                w(s_mm, (8 if merge_mm else NT) * R)
                return
            for r in range(R):
                if compute_only and r >= 1:
                    break
                for a in range(nchunks):
                    if r >= 2:
                        if dma_only:
                            w(s_st[a][r % 2], 16 * (r // 2))
                        else:
                            # WAR: rep r-2's multiplies consumed lt3[a][r%2]
                            mult_done_waits(w, NT * (r - 2) + tpc * (a + 1))
                    sync.dma_start(
                        out=lt3[a][r % 2][:].rearrange(
                            "p (t w) -> p t w", t=tm),
                        in_=loss_v[a],
                    ).then_inc(s_ld[a][r % 2], 16)
            if compute_only:
                w(s_ttg, n_even(NT * R))
                w(s_tt, n_odd(NT * R))
            else:
                for a in range(nchunks):
                    w(s_st[a][0], 16 * ((R + 1) // 2))
                    if R >= 2:
                        w(s_st[a][1], 16 * (R // 2))

        @block.gpsimd
        def _(gpsimd):
            if dma_only:
                return
            # independent ops first (Q7 cores give no same-engine
            # ordering); the mults below self-chain via s_ttg.
            gpsimd.dma_start(out=bb[0:64, :], in_=bboxes[:]).then_inc(s_bb, 16)
            gpsimd.dma_start(out=bb[64:128, :], in_=bboxes[:]).then_inc(s_bb, 16)
            nc.gpsimd.iota(iof[:], [[1, NF]], channel_multiplier=0,
                           allow_small_or_imprecise_dtypes=True
                           ).then_inc(s_io, 1)
            if pe_only:
                return
            w = make_waiter(gpsimd)
            ng = 0
            for g in range(NT * R):
                if g % 2 != 0:
                    continue
                r, j = g // NT, g % NT
                a = _chunk(j)
                if compute_only:
                    w(s_ld[a][0], 16)
                else:
                    w(s_ld[a][r % 2], 16 * (r // 2 + 1))
                if r >= 2 and not compute_only:
                    w(s_st[a][r % 2], 16 * (r // 2))  # ot[a][r%2] stored
                fw_done_wait(w, g)
                if ng >= 1:
                    w(s_ttg, ng)  # self-chain (Q7 in-order)
                nc.gpsimd.tensor_tensor(
                    out=ot[a][0 if compute_only else r % 2][:, _cols(j)],
                    in0=fw[:, (g % 8) * NF:(g % 8 + 1) * NF],
                    in1=lt3[a][0 if compute_only else r % 2][:, _cols(j)],
                    op=mult,
                ).then_inc(s_ttg, 1)
                ng += 1

        @block.tensor
        def _(tensor):
            if dma_only:
                return
            w = make_waiter(tensor)
            w(s_prep, 14)
            if merge_mm:
                # one matmul per tile-PAIR: shared lhsT, [64, 2*NF] rhs,
                # [128, 2*NF] out spanning a PSUM bank pair. Halves the
                # PE instruction count and weight loads.
                for M in range(8 * R):
                    m = M % 8
                    h, x = m // 4, m % 4
                    if M >= 4 and not pe_only:
                        # bank-pair reuse: both crossings of pair M-4
                        w(s_f, M - 3)
                        w(s_fd, M - 3)
                    nc.tensor.matmul(
                        out=cntP[M % 4][:],
                        lhsT=Rp[64 * h:64 * h + 64, x * P:(x + 1) * P],
                        rhs=CC[64 * h:64 * h + 64, :],
                        start=True, stop=True,
                        skip_group_check=pe_only,
                    ).then_inc(s_mm, 1)
            else:
                for g in range(NT * R):
                    idx = g % NT
                    m, n2 = idx // 2, idx % 2
                    h, x = m // 4, m % 4
                    if g >= 8 and not pe_only:
                        # PSUM bank reuse: wait for the fw-producer's read
                        fw_done_wait(w, g - 8)
                    nc.tensor.matmul(
                        out=cnt_ap(g),
                        lhsT=Rp[64 * h:64 * h + 64, x * P:(x + 1) * P],
                        rhs=CC[64 * h:64 * h + 64, n2 * NF:(n2 + 1) * NF],
                        start=True, stop=True,
                        skip_group_check=pe_only,
                    ).then_inc(s_mm, 1)

        @block.vector
        def _(vector):
            if dma_only:
                return
            w = make_waiter(vector)
            # --- indicator prep (sem-chained: same-engine RAW needs
            # sems; TS scalar operands prefetch at issue) ---------------
            nc.vector.memset(biasT[:], -LN9).then_inc(s_prep, 1)
            w(s_bb, 32)
            nc.vector.tensor_copy(out=bbfA[:], in_=bb[:]).then_inc(s_prep, 1)
            w(s_prep, 2)
            nc.vector.tensor_scalar(out=bbfS[:], in0=bbfA[:],
                                    scalar1=-512.0, scalar2=None,
                                    op0=add).then_inc(s_prep, 1)
            w(s_prep, 3)
            nc.vector.tensor_copy(out=bbfR[0:64, :],
                                  in_=bbfA[0:64, :]).then_inc(s_prep, 1)
            nc.vector.tensor_copy(out=bbfR[64:128, :],
                                  in_=bbfS[64:128, :]).then_inc(s_prep, 1)
            w(s_io, 1)
            w(s_prep, 5)
            # R[64h+n, c] = (512h+c >= y1[n]) & (512h+c < y2[n])
            nc.vector.tensor_scalar(out=rge[:], in0=iof[:],
                                    scalar1=bbfR[:, 1:2], scalar2=None,
                                    op0=ge).then_inc(s_prep, 1)
            nc.vector.tensor_scalar(out=rlt[:], in0=iof[:],
                                    scalar1=bbfR[:, 3:4], scalar2=None,
                                    op0=lt_op).then_inc(s_prep, 1)
            w(s_prep, 7)
            nc.vector.tensor_tensor(out=Rp[:], in0=rge[:], in1=rlt[:],
                                    op=mult).then_inc(s_prep, 1)
            # C0[64h+n, c] = (c >= x1[n]) & (c < x2[n])     (both halves)
            nc.vector.tensor_scalar(out=cge[:], in0=iof[:],
                                    scalar1=bbfA[:, 0:1], scalar2=None,
                                    op0=ge).then_inc(s_prep, 1)
            nc.vector.tensor_scalar(out=clt[:], in0=iof[:],
                                    scalar1=bbfA[:, 2:3], scalar2=None,
                                    op0=lt_op).then_inc(s_prep, 1)
            w(s_prep, 10)
            nc.vector.tensor_tensor(out=CC[:, 0:NF], in0=cge[:], in1=clt[:],
                                    op=mult).then_inc(s_prep, 1)
            # C1[64h+n, c] = (512+c >= x1[n]) & (512+c < x2[n])
            nc.vector.tensor_scalar(out=cge1[:], in0=iof[:],
                                    scalar1=bbfS[:, 0:1], scalar2=None,
                                    op0=ge).then_inc(s_prep, 1)
            nc.vector.tensor_scalar(out=clt1[:], in0=iof[:],
                                    scalar1=bbfS[:, 2:3], scalar2=None,
                                    op0=lt_op).then_inc(s_prep, 1)
            w(s_prep, 13)
            nc.vector.tensor_tensor(out=CC[:, NF:2 * NF], in0=cge1[:], in1=clt1[:],
                                    op=mult).then_inc(s_prep, 1)

            if pe_only:
                return
            # --- main stream: odd-tile fw + odd-tile mults -------------
            def emit_tt(g):
                r, j = g // NT, g % NT
                a = _chunk(j)
                if compute_only:
                    w(s_ld[a][0], 16)
                else:
                    w(s_ld[a][r % 2], 16 * (r // 2 + 1))
                    if r >= 2:
                        w(s_st[a][r % 2], 16 * (r // 2))
                w(s_fd, g // 2 + 1)
                nc.vector.tensor_tensor(
                    out=ot[a][0 if compute_only else r % 2][:, _cols(j)],
                    in0=fw[:, (g % 8) * NF:(g % 8 + 1) * NF],
                    in1=lt3[a][0 if compute_only else r % 2][:, _cols(j)],
                    op=mult,
                ).then_inc(s_tt, 1)

            for g in range(NT * R):
                if g % 2 != 1:
                    continue
                if g >= 8:
                    # fw slot reuse: mult of g-8 (odd) must have read it
                    w(s_tt, n_odd(g - 8) + 1)
                w(s_mm, (g // 2 + 1) if merge_mm else (g + 1))
                nc.vector.tensor_scalar(
                    out=fw[:, (g % 8) * NF:(g % 8 + 1) * NF],
                    in0=cnt_ap(g),
                    scalar1=1.0, scalar2=0.1,
                    op0=min_op, op1=max_op,
                ).then_inc(s_fd, 1)
                if g >= 2:
                    emit_tt(g - 2)
            if NT * R >= 1:
                emit_tt(NT * R - 1)

        @block.scalar
        def _(scalar):
            if pe_only:
                return
            w = make_waiter(scalar)
            stores_done = 0

            def emit_store(k):
                r, a = k // nchunks, k % nchunks
                if dma_only:
                    if r >= 2:
                        w(s_st[a][r % 2], 16 * (r // 2))
                    w(s_ld[a][r % 2], 16 * (r // 2 + 1))
                    src = lt3[a][r % 2]
                else:
                    mult_done_waits(w, NT * r + tpc * (a + 1))
                    src = ot[a][r % 2]
                scalar.dma_start(
                    out=out_v[a],
                    in_=src[:].rearrange("p (t w) -> p t w", t=tm),
                ).then_inc(s_st[a][r % 2], 16)

            if dma_only:
                for k in range(nchunks * R):
                    emit_store(k)
                return

            def store_pos(k):  # ACT stream position to emit store k at
                r, a = k // nchunks, k % nchunks
                return NT * r + tpc * (a + 1) + 3

            # dependency-free warmup: loads the Sigmoid LUT set (~2.7us)
            # during the DMA ramp instead of on the first real fw op
            nc.scalar.activation(
                warm[:], warm[:],
                mybir.ActivationFunctionType.Sigmoid, bias=0.0, scale=0.0)

            # --- main stream: even-tile fw (+ store issue) -------------
            w(s_prep, 1)  # biasT memset
            for g in range(NT * R):
                if g % 2 != 0:
                    continue
                if g >= 8:
                    # fw slot reuse: mult of g-8 (even) must have read it
                    w(s_ttg, n_even(g - 8) + 1)
                w(s_mm, (g // 2 + 1) if merge_mm else (g + 1))
                nc.scalar.activation(
                    fw[:, (g % 8) * NF:(g % 8 + 1) * NF],
                    cnt_ap(g),
                    mybir.ActivationFunctionType.Sigmoid,
                    bias=biasT[:, 0:1], scale=BALANCE,
                ).then_inc(s_f, 1)
                if compute_only:
                    continue
                while (stores_done < nchunks * R
                       and g >= store_pos(stores_done)):
                    emit_store(stores_done)
                    stores_done += 1
            while not compute_only and stores_done < nchunks * R:
                emit_store(stores_done)
                stores_done += 1

    return nc


_PROGRAM = None


def kernel(loss, pred_densities, bboxes):
    global _PROGRAM
    import ml_dtypes
    if _PROGRAM is None:
        _PROGRAM = build_program()
    loss = np.ascontiguousarray(
        (np.asarray(loss, dtype=np.float32) * np.float32(PRESCALE))
        .astype(ml_dtypes.bfloat16))
    bboxes = np.ascontiguousarray(np.asarray(bboxes, dtype=np.int32))
    assert loss.shape == (B, H, W) and bboxes.shape == (B, N, 4)
    in_maps = [{"loss": loss[i], "bboxes": bboxes[i]} for i in range(B)]
    res = run_bass_kernel_spmd(_PROGRAM, in_maps, list(range(B)))
    return np.stack([r["out"] for r in res.results], axis=0).astype(np.float32)
